# revision 34
# baseline (speedup 1.0000x reference)
"""Trainium2 Bass kernel for nn_BidirectionalAttention (B=2, N=2048, D=2048, H=16).

Head-parallel tensor sharding across 8 NeuronCores (2 heads/core), bf16
matmul pipeline (fp32 PSUM accumulation):

  phase A: qkv projection from x^T (x chunks stationary, w moving), rope on
           natural layout via DVE, PE-transpose q,k into [head_dim, seq]
           (transposes delayed one tile so PE never waits on DVE rope);
           v stays SBUF-resident in natural [seq, dh] layout with a ones
           column appended per head. The last two tiles run on a slimmer
           PSUM ring set so phase B's first score groups overlap them.
  phase B: per (batch, head): transposed scores s^T[k,q] = k^T.T @ q^T,
           exp on ScalarE -> probs (bf16). AV uses probs pieces as the
           STATIONARY operand against rhs [v | ones] so each [128q x 129]
           psum accumulates both attn@v (natural layout) and the softmax
           denominator in column 128 -- no separate ones-matmul sum pass.
           Scale by fast reciprocal (per-partition scalar broadcast), then
           PE-transpose av into [dh, seq] for the output projection. The
           two heads are interleaved per q-group so the ScalarE exp stream
           overlaps the other head's PE work.
  phase C: output projection partial = av^T.T @ wo_rows, interleaved with
           phase B per batch; partials DMA'd out in bf16 (copies on DVE --
           ScalarE is saturated by the exp stream).
Host: shard/pre-tile/bf16-round inputs (partition-major DRAM layouts keep
every DMA at ~128 long contiguous descriptors), sum the 8 partial outputs
in fp32 (the "all-reduce after wo" done at gather time).
"""

import sys

sys.path.insert(0, "/opt/trn_rl_repo")

import numpy as np
import ml_dtypes

B, SEQ, DIM, NHEAD, DH = 2, 2048, 2048, 16, 128
HL = NHEAD // 8  # heads per core = 2
NCORES = 8
NT = B * SEQ  # 4096 flattened rows
SCALE = 1.0 / np.sqrt(DH)

_PROG = {}


def _build():
    import concourse.tile as tile
    from concourse import bacc, mybir

    f32 = mybir.dt.float32
    bf = mybir.dt.bfloat16
    Exp = mybir.ActivationFunctionType.Exp

    nc = bacc.Bacc("TRN2", target_bir_lowering=False, debug=False, num_devices=NCORES)

    xtt_d = nc.dram_tensor("xtt", [32, 128, 16, 128], bf, kind="ExternalInput")
    wqk_d = nc.dram_tensor("wqkt", [128, 16, 4 * DH], bf, kind="ExternalInput")
    wv_d = nc.dram_tensor("wvt", [128, 16, HL * DH], bf, kind="ExternalInput")
    wo_d = nc.dram_tensor("wot", [128, HL, DIM], bf, kind="ExternalInput")
    cos_d = nc.dram_tensor("cos2", [128, 32 * 128], bf, kind="ExternalInput")
    sin_d = nc.dram_tensor("sin2", [128, 32 * 128], bf, kind="ExternalInput")
    ident_d = nc.dram_tensor("ident", [128, 128], bf, kind="ExternalInput")
    out_d = nc.dram_tensor("out_p", [NT, DIM], bf, kind="ExternalOutput")

    with tile.TileContext(nc) as tc:
        with (
            nc.allow_low_precision(reason="bf16 matmul pipeline, fp32 accumulation"),
            tc.tile_pool(name="const", bufs=1) as cp,
            tc.tile_pool(name="axs", bufs=4) as axs,
            tc.tile_pool(name="awork", bufs=2) as aw,
            tc.tile_pool(name="bprobs", bufs=1) as bp,
            tc.tile_pool(name="bwork", bufs=2) as bw,
            tc.tile_pool(name="cot", bufs=2) as cot,
        ):
            ident = cp.tile([128, 128], bf)
            # q^T / k^T SBUF-resident: [tensor t][128 dh, NT]  (q0 q1 k0 k1)
            qkt_res = [
                cp.tile([128, NT], bf, name=f"qktres{t}", tag=f"qktres{t}")
                for t in range(4)
            ]
            # v natural layout, SBUF-resident: [128 k, 32 rowchunk, 2 head, 129]
            # col 128 of each head slot is the ones column for the denominator.
            v_all = cp.tile([128, 32, HL, 129], bf)
            # av^T per (b, j): [128 dh, SEQ]
            avres = {
                (b, j): cp.tile([128, SEQ], bf, name=f"avres{b}{j}", tag=f"avres{b}{j}")
                for b in range(B)
                for j in range(HL)
            }
            wqk_sb = cp.tile([128, 16, 4 * DH], bf)
            wv_sb = cp.tile([128, 16, HL * DH], bf)
            wo_sb = cp.tile([128, HL, DIM], bf)
            cos_all = cp.tile([128, 32, 128], bf)
            sin_all = cp.tile([128, 32, 128], bf)

            nc.vector.memset(v_all[:, :, :, 128:129], 1.0)

            # ---------------- phase A machinery -----------------------------
            xs_tiles = {}

            def fetch(i):
                t = axs.tile([128, 16, 128], bf, tag="xs", bufs=4, name="xs")
                nc.sync.dma_start(t, xtt_d[i])
                xs_tiles[i] = t

            # first-needed bytes first: tile 0's x quarters interleaved with
            # wqk quarters so the first accumulation chain starts as soon as
            # the leading chunks land. (DMA issue on the sync queue costs
            # ~600ns each; all sources are pre-tiled partition-major.)
            xs_t0 = axs.tile([128, 16, 128], bf, tag="xs", bufs=4, name="xs")
            xs_tiles[0] = xs_t0
            for q in range(4):
                nc.sync.dma_start(
                    xs_t0[:, 4 * q : 4 * q + 4, :], xtt_d[0, :, 4 * q : 4 * q + 4, :]
                )
                nc.sync.dma_start(
                    wqk_sb[:, 4 * q : 4 * q + 4, :], wqk_d[:, 4 * q : 4 * q + 4, :]
                )
            fetch(1)
            nc.sync.dma_start(wv_sb, wv_d[:, :, :])
            fetch(2)
            fetch(3)
            nc.sync.dma_start(ident, ident_d[:, :])
            nc.sync.dma_start(cos_all.rearrange("p i f -> p (i f)"), cos_d[:, :])
            nc.sync.dma_start(sin_all.rearrange("p i f -> p (i f)"), sin_d[:, :])
            nc.sync.dma_start(wo_sb, wo_d[:, :, :])

            pools = {}  # current-scope psum pools/ring depths for emit_tile
            pend = [None]  # (rt tile, g0) awaiting transpose; delayed 1 tile

            def emit_transposes(rt, g0):
                tp = pools["atp"].tile([128, 4, 128], bf, tag="tp", bufs=pools["tpb"])
                for t in range(4):
                    nc.tensor.transpose(
                        tp[:, t, :], rt[:, t * 128 : (t + 1) * 128], ident
                    )
                for t in range(4):
                    nc.scalar.copy(qkt_res[t][:, g0 : g0 + 128], tp[:, t, :])

            def emit_tile(i):
                g0 = i * 128
                xs = xs_tiles.pop(i)
                if i + 4 < 32:  # prefetch, ring depth 4
                    fetch(i + 4)
                qkps = pools["aps"].tile(
                    [128, 4 * DH], f32, tag="qk", bufs=pools["qkb"]
                )
                for cc in range(16):
                    nc.tensor.matmul(
                        qkps,
                        xs[:, cc, :],
                        wqk_sb[:, cc, :],
                        start=(cc == 0),
                        stop=(cc == 15),
                    )
                vps = pools["aps"].tile(
                    [128, HL * DH], f32, tag="v", bufs=pools["vb"]
                )
                for cc in range(16):
                    nc.tensor.matmul(
                        vps,
                        xs[:, cc, :],
                        wv_sb[:, cc, :],
                        start=(cc == 0),
                        stop=(cc == 15),
                    )
                if pend[0] is not None:
                    emit_transposes(*pend[0])
                # v -> v_all natural layout (per-head slots, cols 0:128)
                nc.scalar.copy(
                    v_all[:, i, :, 0:128], vps.rearrange("p (j d) -> p j d", j=HL)
                )

                # rope: cols [q0 q1 k0 k1], each 128 = [64 even | 64 odd]
                rt = aw.tile([128, 4 * DH], bf, tag="rt", bufs=2)
                ca = cos_all[:, i, :].rearrange("p (t f) -> p t f", t=2)
                sa = sin_all[:, i, :].rearrange("p (t f) -> p t f", t=2)
                for g in range(2):
                    blk = qkps[:, g * 256 : (g + 1) * 256].rearrange(
                        "p (t h f) -> p t h f", t=2, h=2
                    )
                    rbl = rt[:, g * 256 : (g + 1) * 256].rearrange(
                        "p (t h f) -> p t h f", t=2, h=2
                    )
                    ev, od = blk[:, :, 0, :], blk[:, :, 1, :]
                    tA = aw.tile([128, 2, 64], bf, tag="tA", bufs=1)
                    tB = aw.tile([128, 2, 64], bf, tag="tB", bufs=1)
                    nc.vector.tensor_mul(tA, od, sa)
                    nc.vector.tensor_mul(tB, ev, ca)
                    nc.vector.tensor_sub(rbl[:, :, 0, :], tB, tA)
                    tC = aw.tile([128, 2, 64], bf, tag="tC", bufs=1)
                    tD = aw.tile([128, 2, 64], bf, tag="tD", bufs=1)
                    nc.vector.tensor_mul(tC, ev, sa)
                    nc.vector.tensor_mul(tD, od, ca)
                    nc.vector.tensor_add(rbl[:, :, 1, :], tD, tC)
                pend[0] = (rt, g0)

            # ---------------- phase A ---------------------------------------
            with (
                tc.tile_pool(name="apsum", bufs=2, space="PSUM") as aps1,
                tc.tile_pool(name="atps", bufs=2, space="PSUM") as atp1,
            ):
                pools.update(aps=aps1, atp=atp1, qkb=3, vb=2, tpb=2)
                for i in range(32):
                    emit_tile(i)
                # tile 31's transposes are deferred into phase B (emitted
                # after the first score groups) so PE never waits on the
                # final rope; its rt tile lives in the outer-scope pool.

            # ---------------- phase B pools + A tail overlap ----------------
            probs_of = {}
            with tc.tile_pool(name="bs", bufs=2, space="PSUM") as bs:

                def emit_scores(b, j, qt):
                    """scores + exp for one 512-wide q group: fills probs."""
                    kt_sb = qkt_res[2 + j][:, b * SEQ : (b + 1) * SEQ]
                    qt_sb = qkt_res[j][:, b * SEQ : (b + 1) * SEQ]
                    q0 = qt * 512
                    probs = bp.tile(
                        [128, 16, 512], bf, tag="probs", bufs=3, name="probs"
                    )
                    probs_of[(b, j, qt)] = probs
                    for kp in range(8):
                        sps = bs.tile([128, 2, 512], f32, tag="s", name="sps")
                        for u in range(2):
                            kt_i = 2 * kp + u
                            nc.tensor.matmul(
                                sps[:, u, :],
                                kt_sb[:, kt_i * 128 : (kt_i + 1) * 128],
                                qt_sb[:, q0 : q0 + 512],
                                start=True,
                                stop=True,
                            )
                        nc.scalar.activation(probs[:, 2 * kp : 2 * kp + 2, :], sps, Exp)

                # ---------- Phase B+C: attention + output projection --------
                with (
                    tc.tile_pool(name="batp", bufs=1, space="PSUM") as batp,
                    tc.tile_pool(name="bav", bufs=3, space="PSUM") as bav,
                ):
                    pend_drain = [None]  # (b, j, qt, [av_s x4]) awaiting drain

                    def flush_drain():
                        """PE-transpose the previous q-group's scaled av into
                        avres; for j==1 groups follow with the C subtiles that
                        consume exactly those columns (both heads done)."""
                        if pend_drain[0] is None:
                            return
                        b, j, qt, av_ss = pend_drain[0]
                        pend_drain[0] = None
                        av_r = avres[(b, j)]
                        avT = batp.tile([128, 4, 128], bf, tag="avT", name="avT")
                        tail = b == 1 and qt == 3
                        for qs in range(4):
                            nc.tensor.transpose(avT[:, qs, :], av_ss[qs], ident)
                            nc.vector.tensor_copy(
                                av_r[
                                    :, qt * 512 + qs * 128 : qt * 512 + (qs + 1) * 128
                                ],
                                avT[:, qs, :],
                            )
                            # tail: interleave C so the final out-DMAs start
                            # early; otherwise emit C after all drains so its
                            # matmuls never wait on the queued DVE copies.
                            if j == 1 and tail:
                                emit_c_subtile(b, 4 * qt + qs, tail=True)
                        if j == 1 and not tail:
                            for qs in range(4):
                                emit_c_subtile(b, 4 * qt + qs, tail=False)

                    def emit_av(b, j, qt):
                        """AV + denominator for one q group (drain deferred)."""
                        probs = probs_of.pop((b, j, qt))
                        avps = []
                        for qsp in range(2):  # pairs of 128-wide q subtiles
                            avp = bav.tile([128, 2, 256], f32, tag="avp", name="avp")
                            avps.append(avp)
                            for u in range(2):
                                qs = 2 * qsp + u
                                for kc in range(16):
                                    nc.tensor.matmul(
                                        avp[:, u, 0:129],
                                        probs[:, kc, qs * 128 : (qs + 1) * 128],
                                        v_all[:, b * 16 + kc, j, :],
                                        start=(kc == 0),
                                        stop=(kc == 15),
                                    )
                        # scales emitted BEFORE flush_drain so the avp-ring
                        # WAR deps of the C-subtile psum grabs see them.
                        av_ss = []
                        for qs in range(4):
                            avp, u = avps[qs // 2], qs % 2
                            rcp = bw.tile([128, 1], f32, tag="rcp", bufs=2, name="rcp")
                            nc.vector.reciprocal_approx_fast(rcp, avp[:, u, 128:129])
                            av_s = bw.tile(
                                [128, 128], bf, tag="avs", bufs=8, name="avs"
                            )
                            nc.vector.tensor_scalar_mul(av_s, avp[:, u, 0:128], rcp)
                            av_ss.append(av_s)
                        flush_drain()
                        pend_drain[0] = (b, j, qt, av_ss)

                    def emit_c_subtile(b, nl, tail):
                        g0 = b * SEQ + nl * 128
                        ot = cot.tile([128, DIM], bf, tag="ot", bufs=4, name="ot")
                        for do in range(4):
                            ops = bav.tile(
                                [128, 512], f32, tag="avp", bufs=3, name="ops"
                            )
                            for j in range(HL):
                                nc.tensor.matmul(
                                    ops,
                                    avres[(b, j)][:, nl * 128 : (nl + 1) * 128],
                                    wo_sb[:, j, do * 512 : (do + 1) * 512],
                                    start=(j == 0),
                                    stop=(j == 1),
                                )
                            osl = ot[:, do * 512 : (do + 1) * 512]
                            # ScalarE is saturated by the exp stream, so the
                            # PSUM->SBUF copies go to DVE; at the tail ACT is
                            # idle, so alternate for speed.
                            if tail and do % 2 == 1:
                                nc.scalar.copy(osl, ops)
                            else:
                                nc.vector.tensor_copy(osl, ops)
                            if tail and do % 2 == 1:
                                nc.sync.dma_start(
                                    out_d[
                                        g0 : g0 + 128, (do - 1) * 512 : (do + 1) * 512
                                    ],
                                    ot[:, (do - 1) * 512 : (do + 1) * 512],
                                )
                        if not tail:
                            nc.sync.dma_start(out_d[g0 : g0 + 128, :], ot)

                    # flush phase A's deferred tile-31 transposes behind the
                    # first score groups (they only need tiles 0..15): the
                    # last rope finishes on DVE under these ~7us of score mms.
                    emit_scores(0, 0, 0)
                    emit_scores(0, 1, 0)
                    rt31, g31 = pend[0]
                    pend[0] = None
                    tpb = batp.tile([128, 4, 128], bf, tag="avT", name="tpb")
                    for t in range(4):
                        nc.tensor.transpose(
                            tpb[:, t, :], rt31[:, t * 128 : (t + 1) * 128], ident
                        )
                    for t in range(4):
                        nc.scalar.copy(qkt_res[t][:, g31 : g31 + 128], tpb[:, t, :])

                    # heads interleaved per q-group; scores one group ahead of
                    # AV; drains (and the j==1 C subtiles) one group further.
                    for b in range(B):
                        if b > 0:
                            emit_scores(b, 0, 0)
                            emit_scores(b, 1, 0)
                        for qt in range(4):
                            if qt < 3:
                                emit_scores(b, 0, qt + 1)
                            emit_av(b, 0, qt)
                            if qt < 3:
                                emit_scores(b, 1, qt + 1)
                            emit_av(b, 1, qt)
                    flush_drain()

    nc.compile()
    return nc


def _get_prog():
    if "prog" not in _PROG:
        _PROG["prog"] = _build()
    return _PROG["prog"], False


def _shard(x, freqs_cis, wqkv, wo, mm_f32r=False):
    bf = ml_dtypes.bfloat16
    x = np.asarray(x, dtype=np.float32)
    freqs_cis = np.asarray(freqs_cis, dtype=np.float32)
    wqkv = np.asarray(wqkv, dtype=np.float32)
    wo = np.asarray(wo, dtype=np.float32)

    # x^T pre-tiled per 128-row output tile: [32 tile, 128 p, 16 chunk, 128]
    xt = x.reshape(NT, DIM).T  # [DIM, NT]
    xtt = np.ascontiguousarray(
        xt.reshape(16, 128, 32, 128).transpose(2, 1, 0, 3)
    ).astype(bf)

    cos = freqs_cis[:, :, 0]  # [SEQ, 64]
    sin = freqs_cis[:, :, 1]
    cosb = np.concatenate([cos] * B, axis=0)  # [NT, 64], row n = b*SEQ + pos
    sinb = np.concatenate([sin] * B, axis=0)
    cos2n = np.concatenate([cosb, cosb], axis=1)  # [NT, 128] dup halves
    sin2n = np.concatenate([sinb, sinb], axis=1)
    # partition-major for contiguous DMA: [128 p, 32 i, 128 f] flattened
    cos2 = cos2n.reshape(32, 128, 128).transpose(1, 0, 2).reshape(128, 32 * 128)
    sin2 = sin2n.reshape(32, 128, 128).transpose(1, 0, 2).reshape(128, 32 * 128)
    cos2 = np.ascontiguousarray(cos2).astype(bf)
    sin2 = np.ascontiguousarray(sin2).astype(bf)

    perm = np.concatenate([np.arange(0, DH, 2), np.arange(1, DH, 2)])  # de-interleave
    ident = np.eye(128, dtype=np.float32).astype(bf)

    def ptile(a, inner):  # [inner*128, m] -> [128 p, inner chunk, m]
        m = a.shape[1]
        return np.ascontiguousarray(
            a.reshape(inner, 128, m).transpose(1, 0, 2)
        ).astype(bf)

    in_maps = []
    for c in range(NCORES):
        h0 = c * HL
        wq = [wqkv[:, h * DH : (h + 1) * DH][:, perm] * SCALE for h in (h0, h0 + 1)]
        wk = [wqkv[:, DIM + h * DH : DIM + (h + 1) * DH][:, perm] for h in (h0, h0 + 1)]
        wqk_c = ptile(np.concatenate(wq + wk, axis=1), 16)  # [128, 16, 512]
        wv_c = ptile(wqkv[:, 2 * DIM + h0 * DH : 2 * DIM + (h0 + HL) * DH], 16)
        wo_c = ptile(wo[h0 * DH : (h0 + HL) * DH, :], HL)  # [128, 2, DIM]
        in_maps.append(
            {
                "xtt": xtt,
                "wqkt": wqk_c,
                "wvt": wv_c,
                "wot": wo_c,
                "cos2": cos2,
                "sin2": sin2,
                "ident": ident,
            }
        )
    return in_maps


def _run(in_maps, trace=False, **kw):
    from concourse.bass_utils import run_bass_kernel_spmd

    prog, _ = _get_prog()
    return run_bass_kernel_spmd(prog, in_maps, list(range(NCORES)), trace=trace, **kw)


def kernel(x, freqs_cis, wqkv, wo):
    _get_prog()
    in_maps = _shard(x, freqs_cis, wqkv, wo)
    res = _run(in_maps, trace=False)
    acc = np.zeros((NT, DIM), dtype=np.float32)
    for c in range(NCORES):
        acc += np.asarray(res.results[c]["out_p"], dtype=np.float32)
    return acc.reshape(B, SEQ, DIM)
